# revision 17
# baseline (speedup 1.0000x reference)
"""Trainium2 Bass kernel for CapsNet dynamic-routing layer.

Problem: B=64, IN_FS=1152, OUT_FS=64, IN_DIM=8, OUT_DIM=16, T=3.
  u_hat = einsum('bfi,fgio->bfgo', x, W)
  b = 0; for T: c = softmax_g(b); s = einsum('bfg,bfgo->bgo', c, u_hat)
           v = squash(s); b += einsum('bfgo,bgo->bfg', u_hat, v)
  return v

Strategy (8 NeuronCores, batch-parallel, B_local=8 per core, no collectives):
 - Host pre-arranges W/x into matmul-friendly fp16 layouts.
 - Pass 1 (per core): stream W tiles, compute u_hat on TensorE via
   block-diagonal x weights; keep u_hat resident in SBUF as fp16 in layout
   [p=(j16,b8), free=(grp72, o16, g64)] (f = 16*grp + j).  A second,
   accumulated matmul against the dense x slice yields s1 = sum_f u_hat
   for free (iteration 1 has uniform coupling c = 1/64).
 - b is never stored: b_k = sum_o u_hat * Vcum with Vcum = sum_{t<k} v_t.
 - Per routing iteration: DVE multiplies u_hat by broadcast Vcum (fp16 2x),
   TensorE reduces over o via identity-weight accumulating matmuls,
   softmax over g runs on [128p, (grp,g)] (ScalarE exp + DVE reduce),
   DVE multiplies u_hat by broadcast c, TensorE contracts over f via a
   b-selector matmul, squash runs on tiny [8,1024] tensors.
"""

import numpy as np
from contextlib import ExitStack

B, IN_FS, OUT_FS, IN_DIM, OUT_DIM = 64, 1152, 64, 8, 16
NCORES = 8
BL = B // NCORES          # 8  batch per core
GRP = IN_FS // 16         # 72 groups of 16 input capsules
NF = GRP * OUT_DIM * OUT_FS  # 73728 free elems per partition for u_hat

_PROGRAM_CACHE = {}


def build_program():
    import concourse.bass as bass
    import concourse.tile as tile
    from concourse import bacc, mybir

    f16 = mybir.dt.float16
    f32 = mybir.dt.float32
    MULT = mybir.AluOpType.mult
    ADD = mybir.AluOpType.add
    AX = mybir.AxisListType.X
    EXP = mybir.ActivationFunctionType.Exp

    nc = bacc.Bacc(
        "TRN2", target_bir_lowering=False, debug=False, num_devices=NCORES
    )

    W2 = nc.dram_tensor("w2", [GRP // 4, 128, 4096], f16, kind="ExternalInput")
    XBD = nc.dram_tensor("xbd", [GRP // 4, 128, 512], f16, kind="ExternalInput")
    XSUM = nc.dram_tensor("xsum", [128, GRP, BL], f16, kind="ExternalInput")
    SEL8 = nc.dram_tensor("sel8", [128, BL], f16, kind="ExternalInput")
    BSEL = nc.dram_tensor("bsel", [BL, 128], f16, kind="ExternalInput")
    I128 = nc.dram_tensor("i128", [128, 128], f16, kind="ExternalInput")
    VOUT = nc.dram_tensor("vout", [BL, 1024], f32, kind="ExternalOutput")

    with tile.TileContext(nc) as tc, ExitStack() as ctx:
        const_pool = ctx.enter_context(tc.tile_pool(name="const", bufs=1))
        u_pool = ctx.enter_context(tc.tile_pool(name="u", bufs=1))
        xbd_pool = ctx.enter_context(tc.tile_pool(name="xbd", bufs=2))
        wp_pool = ctx.enter_context(tc.tile_pool(name="wp", bufs=3))
        bk_pool = ctx.enter_context(tc.tile_pool(name="bk", bufs=1))
        ec_pool = ctx.enter_context(tc.tile_pool(name="ec", bufs=1))
        sm_pool = ctx.enter_context(tc.tile_pool(name="sm", bufs=1))
        pA = ctx.enter_context(tc.tile_pool(name="pA", bufs=6, space="PSUM"))
        pB = ctx.enter_context(tc.tile_pool(name="pB", bufs=1, space="PSUM"))

        # ---- resident constants ----
        sel8_sb = const_pool.tile([128, BL], f16, tag="sel8")
        nc.sync.dma_start(sel8_sb[:, :], SEL8[:, :])
        bsel_sb = const_pool.tile([BL, 128], f16, tag="bsel")
        nc.sync.dma_start(bsel_sb[:, :], BSEL[:, :])
        i128_sb = const_pool.tile([128, 128], f16, tag="i128")
        nc.sync.dma_start(i128_sb[:, :], I128[:, :])
        xsum_sb = const_pool.tile([128, GRP, BL], f16, tag="xsum")
        nc.sync.dma_start(xsum_sb[:, :, :], XSUM[:, :, :])

        # ---- resident u_hat, fp16: [p=(j,b), (grp, o, g)] ----
        u_sb = u_pool.tile([128, GRP, OUT_DIM, OUT_FS], f16, tag="u")

        # ---- small per-iteration tensors ----
        bk_sb = bk_pool.tile([128, GRP, OUT_FS], f16, tag="bk")
        ec_sb = ec_pool.tile([128, GRP, OUT_FS], f16, tag="ec")
        den = sm_pool.tile([128, GRP], f32, tag="den")
        rden = sm_pool.tile([128, GRP], f32, tag="rden")
        vbc = sm_pool.tile([128, 1024], f16, tag="vbc")
        vcum = sm_pool.tile([BL, 1024], f16, tag="vcum")

        def squash(s_psum_ap, scale, v_tag):
            """v = squash(scale * s_psum) on [8, (o,g)] fp32; returns v tile."""
            s = sm_pool.tile([BL, 1024], f32, tag="sq_s")
            nc.scalar.mul(s[:, :], s_psum_ap, scale)
            sqt = sm_pool.tile([BL, 1024], f32, tag="sq_sqt")
            nc.scalar.square(sqt[:, :], s[:, :])
            sq = sm_pool.tile([BL, OUT_FS], f32, tag="sq_sq")
            # reduce over o (stride 64) with o innermost in the AP
            nc.vector.tensor_reduce(
                sq[:, :],
                sqt[:, :].rearrange("p (o g) -> p g o", o=OUT_DIM),
                axis=AX,
                op=ADD,
            )
            nrm = sm_pool.tile([BL, OUT_FS], f32, tag="sq_nrm")
            nc.scalar.sqrt(nrm[:, :], sq[:, :])
            dn = sm_pool.tile([BL, OUT_FS], f32, tag="sq_dn")
            nc.vector.tensor_add(dn[:, :], nrm[:, :], sq[:, :])
            rd = sm_pool.tile([BL, OUT_FS], f32, tag="sq_rd")
            nc.vector.reciprocal(rd[:, :], dn[:, :])
            fac = sm_pool.tile([BL, OUT_FS], f32, tag="sq_fac")
            nc.vector.tensor_mul(fac[:, :], sq[:, :], rd[:, :])
            v = sm_pool.tile([BL, 1024], f32, tag="sq_v")
            nc.vector.tensor_tensor(
                v[:, :].rearrange("p (o g) -> p o g", o=OUT_DIM),
                s[:, :].rearrange("p (o g) -> p o g", o=OUT_DIM),
                fac[:, :].unsqueeze(1).broadcast_to([BL, OUT_DIM, OUT_FS]),
                op=MULT,
            )
            return v

        # =============== pass 1: u_hat + s1 ===============
        # W2 DMAs in 2-group tiles (sync queue), XBD in 4-group tiles
        # (gpsimd queue) — few, large, contiguous transfers.
        ps1 = pB.tile([BL, 1024], f32, tag="acc")
        for gq in range(GRP // 4):
            xbdt = xbd_pool.tile([128, 4, 128], f16, tag="xbd")
            nc.gpsimd.dma_start(
                xbdt[:, :, :].rearrange("p a c -> p (a c)"), XBD[gq, :, :]
            )
            w2t = wp_pool.tile([128, 4096], f16, tag="wp")
            nc.sync.dma_start(w2t[:, :], W2[gq, :, :])
            for gw in range(2):
                for k in range(2):
                    grp = gq * 4 + gw * 2 + k
                    w2s = w2t[:, (gw * 2 + k) * 1024:(gw * 2 + k + 1) * 1024]
                    pu0 = pA.tile([128, 512], f32, tag="mm512")
                    pu1 = pA.tile([128, 512], f32, tag="mm512")
                    for h, pu in enumerate((pu0, pu1)):
                        nc.tensor.matmul(
                            pu[:, :], lhsT=xbdt[:, 2 * gw + k, :],
                            rhs=w2s[:, h * 512:(h + 1) * 512],
                            start=True, stop=True,
                        )
                    for h in range(2):
                        nc.tensor.matmul(
                            ps1[:, h * 512:(h + 1) * 512],
                            lhsT=xsum_sb[:, grp, :],
                            rhs=w2s[:, h * 512:(h + 1) * 512],
                            start=(grp == 0), stop=(grp == GRP - 1),
                        )
                    ug = u_sb[:, grp, :, :].rearrange("p o g -> p (o g)")
                    nc.vector.tensor_copy(ug[:, 0:384], pu0[:, 0:384])
                    nc.scalar.copy(ug[:, 384:512], pu0[:, 384:512])
                    nc.scalar.copy(ug[:, 512:1024], pu1[:, :])

        # =============== iteration 1 (c uniform = 1/64) ===============
        v1 = squash(ps1[:, :], 1.0 / OUT_FS, "v1")
        nc.vector.tensor_copy(vcum[:, :], v1[:, :])

        # =============== iterations 2..T ===============
        for it in (2, 3):
            sfx = f"_i{it}"
            # --- broadcast Vcum to all 128 partitions (fp16) ---
            pv0 = pA.tile([128, 512], f32, tag="mm512")
            pv1 = pA.tile([128, 512], f32, tag="mm512")
            for h, pv in enumerate((pv0, pv1)):
                nc.tensor.matmul(
                    pv[:, :], lhsT=bsel_sb[:, :],
                    rhs=vcum[:, h * 512:(h + 1) * 512],
                    start=True, stop=True,
                )
            nc.vector.tensor_copy(vbc[:, 0:512], pv0[:, :])
            nc.scalar.copy(vbc[:, 512:1024], pv1[:, :])
            vbc3 = vbc[:, :].rearrange("p (o g) -> p o g", o=OUT_DIM)

            # --- phase A: b_k = sum_o u*Vcum  (9 blocks x 8 grps) ---
            vbc4 = vbc3.unsqueeze(1).broadcast_to([128, 2, OUT_DIM, OUT_FS])
            for blk in range(9):
                pbk = pA.tile([128, 512], f32, tag="mm512")
                for q in range(2):
                    g0 = blk * 8 + q * 4
                    w4 = wp_pool.tile([128, 4, OUT_DIM, OUT_FS], f16, tag="wp")
                    for h in range(2):
                        nc.vector.tensor_tensor(
                            w4[:, 2 * h:2 * h + 2, :, :],
                            u_sb[:, g0 + 2 * h:g0 + 2 * h + 2, :, :],
                            vbc4, op=MULT,
                        )
                    for o in range(OUT_DIM):
                        nc.tensor.matmul(
                            pbk[:, q * 256:(q + 1) * 256],
                            lhsT=i128_sb[:, :], rhs=w4[:, :, o, :],
                            start=(o == 0), stop=(o == OUT_DIM - 1),
                        )
                bco = bk_sb[:, blk * 8:(blk + 1) * 8, :].rearrange("p a g -> p (a g)")
                nc.scalar.copy(bco[:, :], pbk[:, :])

            # --- softmax over g (free minor), chunked by grp-halves so it
            # overlaps phase A's tail and phase B's head ---
            def softmax_chunk(glo, ghi):
                n = ghi - glo
                nc.scalar.activation(
                    ec_sb[:, glo:ghi, :].rearrange("p a g -> p (a g)"),
                    bk_sb[:, glo:ghi, :].rearrange("p a g -> p (a g)"),
                    EXP,
                )
                nc.vector.tensor_reduce(
                    den[:, glo:ghi], ec_sb[:, glo:ghi, :], axis=AX, op=ADD
                )
                nc.vector.reciprocal(rden[:, glo:ghi], den[:, glo:ghi])
                nc.vector.tensor_tensor(
                    ec_sb[:, glo:ghi, :], ec_sb[:, glo:ghi, :],
                    rden[:, glo:ghi].unsqueeze(2).broadcast_to(
                        [128, n, OUT_FS]
                    ),
                    op=MULT,
                )

            # --- phase B: s_k = sum_f c*u (pairs of grps) ---
            ps = pB.tile([BL, 1024], f32, tag="acc")

            def phase_b(gp_lo, gp_hi):
                for gp in range(gp_lo, gp_hi):
                    pc = wp_pool.tile([128, 2, OUT_DIM, OUT_FS], f16, tag="wp")
                    nc.vector.tensor_tensor(
                        pc[:, :, :, :],
                        u_sb[:, 2 * gp:2 * gp + 2, :, :],
                        ec_sb[:, 2 * gp:2 * gp + 2, :].unsqueeze(2).broadcast_to(
                            [128, 2, OUT_DIM, OUT_FS]
                        ),
                        op=MULT,
                    )
                    pcf = pc[:, :, :, :].rearrange("p a o g -> p (a o g)")
                    for h in range(4):
                        nc.tensor.matmul(
                            ps[:, (h % 2) * 512:(h % 2) * 512 + 512],
                            lhsT=sel8_sb[:, :],
                            rhs=pcf[:, h * 512:(h + 1) * 512],
                            start=(gp == 0 and h < 2),
                            stop=(gp == GRP // 2 - 1 and h >= 2),
                        )

            softmax_chunk(0, 32)
            phase_b(0, 16)
            softmax_chunk(32, GRP)
            phase_b(16, GRP // 2)

            v = squash(ps[:, :], 1.0, "v" + sfx)
            if it < 3:
                nc.vector.tensor_add(vcum[:, :], vcum[:, :], v[:, :])
            else:
                nc.sync.dma_start(VOUT[:, :], v[:, :])

    nc.finalize()
    return nc


def prepare_inputs(x, W):
    """Host-side layout prep. Returns (shared_map, [per-core maps])."""
    f16 = np.float16
    # W2[grp, 8j+i, 64o+g] = W[16grp+j, g, i, o]
    W2 = np.ascontiguousarray(
        W.astype(np.float32).reshape(GRP, 16, OUT_FS, IN_DIM, OUT_DIM)
        .transpose(0, 1, 3, 4, 2).reshape(GRP, 128, 1024)
    ).astype(f16)
    # pack W2 into 4-group DMA tiles: [18, 128, 4096]
    W2 = np.ascontiguousarray(
        W2.reshape(GRP // 4, 4, 128, 1024).transpose(0, 2, 1, 3)
        .reshape(GRP // 4, 128, 4096)
    )
    SEL8 = np.tile(np.eye(BL, dtype=f16), (16, 1))            # [128, 8]
    BSEL = np.tile(np.eye(BL, dtype=f16), (1, 16))            # [8, 128]
    I128 = np.eye(128, dtype=f16)

    shared = {"w2": W2, "sel8": SEL8, "bsel": BSEL, "i128": I128}
    per_core = []
    for ci in range(NCORES):
        xc = np.asarray(x[ci * BL:(ci + 1) * BL], dtype=np.float32)
        xr = xc.transpose(1, 2, 0).reshape(GRP, 16, IN_DIM, BL)  # [grp,j,i,b]
        xbd = np.zeros((GRP, 16, IN_DIM, 16, BL), dtype=f16)
        for j in range(16):
            xbd[:, j, :, j, :] = xr[:, j]
        xbd = xbd.reshape(GRP, 128, 128)
        # pack into 4-group DMA tiles: [18, 128, 512]
        xbd = np.ascontiguousarray(
            xbd.reshape(GRP // 4, 4, 128, 128).transpose(0, 2, 1, 3)
            .reshape(GRP // 4, 128, 512)
        )
        xsum = np.ascontiguousarray(
            xr.transpose(1, 2, 0, 3).reshape(128, GRP, BL)
        ).astype(f16)
        m = dict(shared)
        m["xbd"] = xbd
        m["xsum"] = xsum
        per_core.append(m)
    return per_core


def kernel(x, W):
    from concourse.bass_utils import run_bass_kernel_spmd

    x = np.asarray(x)
    W = np.asarray(W)
    if "nc" not in _PROGRAM_CACHE:
        _PROGRAM_CACHE["nc"] = build_program()
    nc = _PROGRAM_CACHE["nc"]
    in_maps = prepare_inputs(x, W)
    res = run_bass_kernel_spmd(nc, in_maps, list(range(NCORES)))
    outs = []
    for ci in range(NCORES):
        v = np.asarray(res.results[ci]["vout"], dtype=np.float32)
        outs.append(v.reshape(BL, OUT_DIM, OUT_FS).transpose(0, 2, 1))
    return np.concatenate(outs, axis=0).astype(np.float32)


# revision 18
# speedup vs baseline: 1.1238x; 1.1238x over previous
"""Trainium2 Bass kernel for CapsNet dynamic-routing layer.

Problem: B=64, IN_FS=1152, OUT_FS=64, IN_DIM=8, OUT_DIM=16, T=3.
  u_hat = einsum('bfi,fgio->bfgo', x, W)
  b = 0; for T: c = softmax_g(b); s = einsum('bfg,bfgo->bgo', c, u_hat)
           v = squash(s); b += einsum('bfgo,bgo->bfg', u_hat, v)
  return v

Strategy (8 NeuronCores, batch-parallel, B_local=8 per core, no collectives):
 - Host pre-arranges W/x into matmul-friendly fp16 layouts.
 - Pass 1 (per core): stream W tiles, compute u_hat on TensorE via
   block-diagonal x weights; keep u_hat resident in SBUF as fp16 in layout
   [p=(j16,b8), free=(grp72, o16, g64)] (f = 16*grp + j).  A second,
   accumulated matmul against the dense x slice yields s1 = sum_f u_hat
   for free (iteration 1 has uniform coupling c = 1/64).
 - b is never stored: b_k = sum_o u_hat * Vcum with Vcum = sum_{t<k} v_t.
 - Per routing iteration: DVE multiplies u_hat by broadcast Vcum (fp16 2x),
   TensorE reduces over o via identity-weight accumulating matmuls,
   softmax over g runs on [128p, (grp,g)] (ScalarE exp + DVE reduce),
   DVE multiplies u_hat by broadcast c, TensorE contracts over f via a
   b-selector matmul, squash runs on tiny [8,1024] tensors.
"""

import numpy as np
from contextlib import ExitStack

B, IN_FS, OUT_FS, IN_DIM, OUT_DIM = 64, 1152, 64, 8, 16
NCORES = 8
BL = B // NCORES          # 8  batch per core
GRP = IN_FS // 16         # 72 groups of 16 input capsules
NF = GRP * OUT_DIM * OUT_FS  # 73728 free elems per partition for u_hat

_PROGRAM_CACHE = {}


def build_program():
    import concourse.bass as bass
    import concourse.tile as tile
    from concourse import bacc, mybir

    f16 = mybir.dt.float16
    f32 = mybir.dt.float32
    MULT = mybir.AluOpType.mult
    ADD = mybir.AluOpType.add
    AX = mybir.AxisListType.X
    EXP = mybir.ActivationFunctionType.Exp

    nc = bacc.Bacc(
        "TRN2", target_bir_lowering=False, debug=False, num_devices=NCORES
    )

    W2 = nc.dram_tensor("w2", [GRP // 4, 128, 4096], f16, kind="ExternalInput")
    XBD = nc.dram_tensor("xbd", [GRP // 4, 128, 512], f16, kind="ExternalInput")
    XSUM = nc.dram_tensor("xsum", [128, GRP, BL], f16, kind="ExternalInput")
    SEL8 = nc.dram_tensor("sel8", [128, BL], f16, kind="ExternalInput")
    BSEL = nc.dram_tensor("bsel", [BL, 128], f16, kind="ExternalInput")
    I128 = nc.dram_tensor("i128", [128, 128], f16, kind="ExternalInput")
    VOUT = nc.dram_tensor("vout", [BL, 1024], f32, kind="ExternalOutput")

    with tile.TileContext(nc) as tc, ExitStack() as ctx:
        const_pool = ctx.enter_context(tc.tile_pool(name="const", bufs=1))
        u_pool = ctx.enter_context(tc.tile_pool(name="u", bufs=1))
        xbd_pool = ctx.enter_context(tc.tile_pool(name="xbd", bufs=2))
        wp_pool = ctx.enter_context(tc.tile_pool(name="wp", bufs=3))
        bk_pool = ctx.enter_context(tc.tile_pool(name="bk", bufs=1))
        ec_pool = ctx.enter_context(tc.tile_pool(name="ec", bufs=1))
        sm_pool = ctx.enter_context(tc.tile_pool(name="sm", bufs=1))
        pA = ctx.enter_context(tc.tile_pool(name="pA", bufs=6, space="PSUM"))
        pB = ctx.enter_context(tc.tile_pool(name="pB", bufs=1, space="PSUM"))

        # ---- resident constants ----
        sel8_sb = const_pool.tile([128, BL], f16, tag="sel8")
        nc.sync.dma_start(sel8_sb[:, :], SEL8[:, :])
        bsel_sb = const_pool.tile([BL, 128], f16, tag="bsel")
        nc.sync.dma_start(bsel_sb[:, :], BSEL[:, :])
        i128_sb = const_pool.tile([128, 128], f16, tag="i128")
        nc.sync.dma_start(i128_sb[:, :], I128[:, :])
        xsum_sb = const_pool.tile([128, GRP, BL], f16, tag="xsum")
        nc.sync.dma_start(xsum_sb[:, :, :], XSUM[:, :, :])

        # ---- resident u_hat, fp16: [p=(j,b), (grp, o, g)] ----
        u_sb = u_pool.tile([128, GRP, OUT_DIM, OUT_FS], f16, tag="u")

        # ---- small per-iteration tensors ----
        bk_sb = bk_pool.tile([128, GRP, OUT_FS], f16, tag="bk")
        ec_sb = ec_pool.tile([128, GRP, OUT_FS], f16, tag="ec")
        den = sm_pool.tile([128, GRP], f32, tag="den")
        rden = sm_pool.tile([128, GRP], f32, tag="rden")
        vbc = sm_pool.tile([128, 1024], f16, tag="vbc")
        vcum = sm_pool.tile([BL, 1024], f16, tag="vcum")

        def squash(s_psum_ap, scale, v_tag):
            """v = squash(scale * s_psum) on [8, (o,g)] fp32; returns v tile."""
            s = sm_pool.tile([BL, 1024], f32, tag="sq_s")
            nc.scalar.mul(s[:, :], s_psum_ap, scale)
            sqt = sm_pool.tile([BL, 1024], f32, tag="sq_sqt")
            nc.scalar.square(sqt[:, :], s[:, :])
            sq = sm_pool.tile([BL, OUT_FS], f32, tag="sq_sq")
            # reduce over o (stride 64) with o innermost in the AP
            nc.vector.tensor_reduce(
                sq[:, :],
                sqt[:, :].rearrange("p (o g) -> p g o", o=OUT_DIM),
                axis=AX,
                op=ADD,
            )
            nrm = sm_pool.tile([BL, OUT_FS], f32, tag="sq_nrm")
            nc.scalar.sqrt(nrm[:, :], sq[:, :])
            dn = sm_pool.tile([BL, OUT_FS], f32, tag="sq_dn")
            nc.vector.tensor_add(dn[:, :], nrm[:, :], sq[:, :])
            rd = sm_pool.tile([BL, OUT_FS], f32, tag="sq_rd")
            nc.vector.reciprocal(rd[:, :], dn[:, :])
            fac = sm_pool.tile([BL, OUT_FS], f32, tag="sq_fac")
            nc.vector.tensor_mul(fac[:, :], sq[:, :], rd[:, :])
            v = sm_pool.tile([BL, 1024], f32, tag="sq_v")
            nc.vector.tensor_tensor(
                v[:, :].rearrange("p (o g) -> p o g", o=OUT_DIM),
                s[:, :].rearrange("p (o g) -> p o g", o=OUT_DIM),
                fac[:, :].unsqueeze(1).broadcast_to([BL, OUT_DIM, OUT_FS]),
                op=MULT,
            )
            return v

        # =============== pass 1: u_hat + s1 ===============
        # W2 DMAs in 2-group tiles (sync queue), XBD in 4-group tiles
        # (gpsimd queue) — few, large, contiguous transfers.
        ps1 = pB.tile([BL, 1024], f32, tag="acc")
        for gq in range(GRP // 4):
            xbdt = xbd_pool.tile([128, 4, 128], f16, tag="xbd")
            nc.gpsimd.dma_start(
                xbdt[:, :, :].rearrange("p a c -> p (a c)"), XBD[gq, :, :]
            )
            w2t = wp_pool.tile([128, 4096], f16, tag="wp")
            nc.sync.dma_start(w2t[:, :], W2[gq, :, :])
            for gw in range(2):
                for k in range(2):
                    grp = gq * 4 + gw * 2 + k
                    w2s = w2t[:, (gw * 2 + k) * 1024:(gw * 2 + k + 1) * 1024]
                    pu0 = pA.tile([128, 512], f32, tag="mm512")
                    pu1 = pA.tile([128, 512], f32, tag="mm512")
                    for h, pu in enumerate((pu0, pu1)):
                        nc.tensor.matmul(
                            pu[:, :], lhsT=xbdt[:, 2 * gw + k, :],
                            rhs=w2s[:, h * 512:(h + 1) * 512],
                            start=True, stop=True,
                        )
                    for h in range(2):
                        nc.tensor.matmul(
                            ps1[:, h * 512:(h + 1) * 512],
                            lhsT=xsum_sb[:, grp, :],
                            rhs=w2s[:, h * 512:(h + 1) * 512],
                            start=(grp == 0), stop=(grp == GRP - 1),
                        )
                    ug = u_sb[:, grp, :, :].rearrange("p o g -> p (o g)")
                    nc.vector.tensor_copy(ug[:, 0:384], pu0[:, 0:384])
                    nc.scalar.copy(ug[:, 384:512], pu0[:, 384:512])
                    nc.scalar.copy(ug[:, 512:1024], pu1[:, :])

        # =============== iteration 1 (c uniform = 1/64) ===============
        v1 = squash(ps1[:, :], 1.0 / OUT_FS, "v1")
        nc.vector.tensor_copy(vcum[:, :], v1[:, :])

        # =============== iterations 2..T ===============
        for it in (2, 3):
            sfx = f"_i{it}"
            # --- broadcast Vcum to all 128 partitions (fp16) ---
            pv0 = pA.tile([128, 512], f32, tag="mm512")
            pv1 = pA.tile([128, 512], f32, tag="mm512")
            for h, pv in enumerate((pv0, pv1)):
                nc.tensor.matmul(
                    pv[:, :], lhsT=bsel_sb[:, :],
                    rhs=vcum[:, h * 512:(h + 1) * 512],
                    start=True, stop=True,
                )
            nc.vector.tensor_copy(vbc[:, 0:512], pv0[:, :])
            nc.scalar.copy(vbc[:, 512:1024], pv1[:, :])
            vbc3 = vbc[:, :].rearrange("p (o g) -> p o g", o=OUT_DIM)

            # --- phase A: b_k = sum_o u*Vcum  (9 blocks x 8 grps) ---
            vbc4 = vbc3.unsqueeze(1).broadcast_to([128, 4, OUT_DIM, OUT_FS])
            for blk in range(9):
                pbk = pA.tile([128, 512], f32, tag="mm512")
                for q in range(2):
                    bq = blk * 2 + q
                    g0 = blk * 8 + q * 4
                    w4 = wp_pool.tile([128, 4, OUT_DIM, OUT_FS], f16, tag="wp")
                    eng = nc.gpsimd if bq % 4 == 3 else nc.vector
                    eng.tensor_tensor(
                        w4[:, :, :, :], u_sb[:, g0:g0 + 4, :, :], vbc4, op=MULT
                    )
                    for o in range(OUT_DIM):
                        nc.tensor.matmul(
                            pbk[:, q * 256:(q + 1) * 256],
                            lhsT=i128_sb[:, :], rhs=w4[:, :, o, :],
                            start=(o == 0), stop=(o == OUT_DIM - 1),
                        )
                bco = bk_sb[:, blk * 8:(blk + 1) * 8, :].rearrange("p a g -> p (a g)")
                nc.scalar.copy(bco[:, :], pbk[:, :])

            # --- softmax over g (free minor), chunked by grp-halves so it
            # overlaps phase A's tail and phase B's head ---
            def softmax_chunk(glo, ghi):
                n = ghi - glo
                nc.scalar.activation(
                    ec_sb[:, glo:ghi, :].rearrange("p a g -> p (a g)"),
                    bk_sb[:, glo:ghi, :].rearrange("p a g -> p (a g)"),
                    EXP,
                )
                nc.vector.tensor_reduce(
                    den[:, glo:ghi], ec_sb[:, glo:ghi, :], axis=AX, op=ADD
                )
                nc.vector.reciprocal(rden[:, glo:ghi], den[:, glo:ghi])
                nc.vector.tensor_tensor(
                    ec_sb[:, glo:ghi, :], ec_sb[:, glo:ghi, :],
                    rden[:, glo:ghi].unsqueeze(2).broadcast_to(
                        [128, n, OUT_FS]
                    ),
                    op=MULT,
                )

            # --- phase B: s_k = sum_f c*u (pairs of grps) ---
            ps = pB.tile([BL, 1024], f32, tag="acc")

            def phase_b(q_lo, q_hi):
                for q in range(q_lo, q_hi):
                    pc = wp_pool.tile([128, 4, OUT_DIM, OUT_FS], f16, tag="wp")
                    eng = nc.gpsimd if q % 4 == 3 else nc.vector
                    eng.tensor_tensor(
                        pc[:, :, :, :],
                        u_sb[:, 4 * q:4 * q + 4, :, :],
                        ec_sb[:, 4 * q:4 * q + 4, :].unsqueeze(2).broadcast_to(
                            [128, 4, OUT_DIM, OUT_FS]
                        ),
                        op=MULT,
                    )
                    pcf = pc[:, :, :, :].rearrange("p a o g -> p (a o g)")
                    for h in range(8):
                        nc.tensor.matmul(
                            ps[:, (h % 2) * 512:(h % 2) * 512 + 512],
                            lhsT=sel8_sb[:, :],
                            rhs=pcf[:, h * 512:(h + 1) * 512],
                            start=(q == 0 and h < 2),
                            stop=(q == GRP // 4 - 1 and h >= 6),
                        )

            softmax_chunk(0, 32)
            phase_b(0, 8)
            softmax_chunk(32, GRP)
            phase_b(8, GRP // 4)

            v = squash(ps[:, :], 1.0, "v" + sfx)
            if it < 3:
                nc.vector.tensor_add(vcum[:, :], vcum[:, :], v[:, :])
            else:
                nc.sync.dma_start(VOUT[:, :], v[:, :])

    nc.finalize()
    return nc


def prepare_inputs(x, W):
    """Host-side layout prep. Returns (shared_map, [per-core maps])."""
    f16 = np.float16
    # W2[grp, 8j+i, 64o+g] = W[16grp+j, g, i, o]
    W2 = np.ascontiguousarray(
        W.astype(np.float32).reshape(GRP, 16, OUT_FS, IN_DIM, OUT_DIM)
        .transpose(0, 1, 3, 4, 2).reshape(GRP, 128, 1024)
    ).astype(f16)
    # pack W2 into 4-group DMA tiles: [18, 128, 4096]
    W2 = np.ascontiguousarray(
        W2.reshape(GRP // 4, 4, 128, 1024).transpose(0, 2, 1, 3)
        .reshape(GRP // 4, 128, 4096)
    )
    SEL8 = np.tile(np.eye(BL, dtype=f16), (16, 1))            # [128, 8]
    BSEL = np.tile(np.eye(BL, dtype=f16), (1, 16))            # [8, 128]
    I128 = np.eye(128, dtype=f16)

    shared = {"w2": W2, "sel8": SEL8, "bsel": BSEL, "i128": I128}
    per_core = []
    for ci in range(NCORES):
        xc = np.asarray(x[ci * BL:(ci + 1) * BL], dtype=np.float32)
        xr = xc.transpose(1, 2, 0).reshape(GRP, 16, IN_DIM, BL)  # [grp,j,i,b]
        xbd = np.zeros((GRP, 16, IN_DIM, 16, BL), dtype=f16)
        for j in range(16):
            xbd[:, j, :, j, :] = xr[:, j]
        xbd = xbd.reshape(GRP, 128, 128)
        # pack into 4-group DMA tiles: [18, 128, 512]
        xbd = np.ascontiguousarray(
            xbd.reshape(GRP // 4, 4, 128, 128).transpose(0, 2, 1, 3)
            .reshape(GRP // 4, 128, 512)
        )
        xsum = np.ascontiguousarray(
            xr.transpose(1, 2, 0, 3).reshape(128, GRP, BL)
        ).astype(f16)
        m = dict(shared)
        m["xbd"] = xbd
        m["xsum"] = xsum
        per_core.append(m)
    return per_core


def kernel(x, W):
    from concourse.bass_utils import run_bass_kernel_spmd

    x = np.asarray(x)
    W = np.asarray(W)
    if "nc" not in _PROGRAM_CACHE:
        _PROGRAM_CACHE["nc"] = build_program()
    nc = _PROGRAM_CACHE["nc"]
    in_maps = prepare_inputs(x, W)
    res = run_bass_kernel_spmd(nc, in_maps, list(range(NCORES)))
    outs = []
    for ci in range(NCORES):
        v = np.asarray(res.results[ci]["vout"], dtype=np.float32)
        outs.append(v.reshape(BL, OUT_DIM, OUT_FS).transpose(0, 2, 1))
    return np.concatenate(outs, axis=0).astype(np.float32)


# revision 19
# speedup vs baseline: 1.1437x; 1.0177x over previous
"""Trainium2 Bass kernel for CapsNet dynamic-routing layer.

Problem: B=64, IN_FS=1152, OUT_FS=64, IN_DIM=8, OUT_DIM=16, T=3.
  u_hat = einsum('bfi,fgio->bfgo', x, W)
  b = 0; for T: c = softmax_g(b); s = einsum('bfg,bfgo->bgo', c, u_hat)
           v = squash(s); b += einsum('bfgo,bgo->bfg', u_hat, v)
  return v

Strategy (8 NeuronCores, batch-parallel, B_local=8 per core, no collectives):
 - Host pre-arranges W/x into matmul-friendly fp16 layouts.
 - Pass 1 (per core): stream W tiles, compute u_hat on TensorE via
   block-diagonal x weights; keep u_hat resident in SBUF as fp16 in layout
   [p=(j16,b8), free=(grp72, o16, g64)] (f = 16*grp + j).  A second,
   accumulated matmul against the dense x slice yields s1 = sum_f u_hat
   for free (iteration 1 has uniform coupling c = 1/64).
 - b is never stored: b_k = sum_o u_hat * Vcum with Vcum = sum_{t<k} v_t.
 - Per routing iteration: DVE multiplies u_hat by broadcast Vcum (fp16 2x),
   TensorE reduces over o via identity-weight accumulating matmuls,
   softmax over g runs on [128p, (grp,g)] (ScalarE exp + DVE reduce),
   DVE multiplies u_hat by broadcast c, TensorE contracts over f via a
   b-selector matmul, squash runs on tiny [8,1024] tensors.
"""

import numpy as np
from contextlib import ExitStack

B, IN_FS, OUT_FS, IN_DIM, OUT_DIM = 64, 1152, 64, 8, 16
NCORES = 8
BL = B // NCORES          # 8  batch per core
GRP = IN_FS // 16         # 72 groups of 16 input capsules
NF = GRP * OUT_DIM * OUT_FS  # 73728 free elems per partition for u_hat

_PROGRAM_CACHE = {}


def build_program():
    import concourse.bass as bass
    import concourse.tile as tile
    from concourse import bacc, mybir

    f16 = mybir.dt.float16
    f32 = mybir.dt.float32
    MULT = mybir.AluOpType.mult
    ADD = mybir.AluOpType.add
    AX = mybir.AxisListType.X
    EXP = mybir.ActivationFunctionType.Exp

    nc = bacc.Bacc(
        "TRN2", target_bir_lowering=False, debug=False, num_devices=NCORES
    )

    W2 = nc.dram_tensor("w2", [GRP // 4, 128, 4096], f16, kind="ExternalInput")
    XBD = nc.dram_tensor("xbd", [GRP // 4, 128, 512], f16, kind="ExternalInput")
    XSUM = nc.dram_tensor("xsum", [128, GRP, BL], f16, kind="ExternalInput")
    SEL8 = nc.dram_tensor("sel8", [128, BL], f16, kind="ExternalInput")
    BSEL = nc.dram_tensor("bsel", [BL, 128], f16, kind="ExternalInput")
    I128 = nc.dram_tensor("i128", [128, 128], f16, kind="ExternalInput")
    VOUT = nc.dram_tensor("vout", [BL, 1024], f32, kind="ExternalOutput")

    with tile.TileContext(nc) as tc, ExitStack() as ctx:
        const_pool = ctx.enter_context(tc.tile_pool(name="const", bufs=1))
        u_pool = ctx.enter_context(tc.tile_pool(name="u", bufs=1))
        xbd_pool = ctx.enter_context(tc.tile_pool(name="xbd", bufs=2))
        wp_pool = ctx.enter_context(tc.tile_pool(name="wp", bufs=4))
        ec_pool = ctx.enter_context(tc.tile_pool(name="ec", bufs=1))
        sm_pool = ctx.enter_context(tc.tile_pool(name="sm", bufs=1))
        pA = ctx.enter_context(tc.tile_pool(name="pA", bufs=6, space="PSUM"))
        pB = ctx.enter_context(tc.tile_pool(name="pB", bufs=1, space="PSUM"))

        # ---- resident constants ----
        sel8_sb = const_pool.tile([128, BL], f16, tag="sel8")
        nc.sync.dma_start(sel8_sb[:, :], SEL8[:, :])
        bsel_sb = const_pool.tile([BL, 128], f16, tag="bsel")
        nc.sync.dma_start(bsel_sb[:, :], BSEL[:, :])
        i128_sb = const_pool.tile([128, 128], f16, tag="i128")
        nc.sync.dma_start(i128_sb[:, :], I128[:, :])
        xsum_sb = const_pool.tile([128, GRP, BL], f16, tag="xsum")
        nc.sync.dma_start(xsum_sb[:, :, :], XSUM[:, :, :])

        # ---- resident u_hat, fp16: [p=(j,b), (grp, o, g)] ----
        u_sb = u_pool.tile([128, GRP, OUT_DIM, OUT_FS], f16, tag="u")

        # ---- small per-iteration tensors ----
        ec_sb = ec_pool.tile([128, GRP, OUT_FS], f16, tag="ec")
        den = sm_pool.tile([128, GRP], f32, tag="den")
        rden = sm_pool.tile([128, GRP], f32, tag="rden")
        vbc = sm_pool.tile([128, 1024], f16, tag="vbc")
        vcum = sm_pool.tile([BL, 1024], f16, tag="vcum")

        def squash(s_psum_ap, scale, v_tag):
            """v = squash(scale * s_psum) on [8, (o,g)] fp32; returns v tile."""
            s = sm_pool.tile([BL, 1024], f32, tag="sq_s")
            nc.scalar.mul(s[:, :], s_psum_ap, scale)
            sqt = sm_pool.tile([BL, 1024], f32, tag="sq_sqt")
            nc.scalar.square(sqt[:, :], s[:, :])
            sq = sm_pool.tile([BL, OUT_FS], f32, tag="sq_sq")
            # reduce over o (stride 64) with o innermost in the AP
            nc.vector.tensor_reduce(
                sq[:, :],
                sqt[:, :].rearrange("p (o g) -> p g o", o=OUT_DIM),
                axis=AX,
                op=ADD,
            )
            nrm = sm_pool.tile([BL, OUT_FS], f32, tag="sq_nrm")
            nc.scalar.sqrt(nrm[:, :], sq[:, :])
            dn = sm_pool.tile([BL, OUT_FS], f32, tag="sq_dn")
            nc.vector.tensor_add(dn[:, :], nrm[:, :], sq[:, :])
            rd = sm_pool.tile([BL, OUT_FS], f32, tag="sq_rd")
            nc.vector.reciprocal(rd[:, :], dn[:, :])
            fac = sm_pool.tile([BL, OUT_FS], f32, tag="sq_fac")
            nc.vector.tensor_mul(fac[:, :], sq[:, :], rd[:, :])
            v = sm_pool.tile([BL, 1024], f32, tag="sq_v")
            nc.vector.tensor_tensor(
                v[:, :].rearrange("p (o g) -> p o g", o=OUT_DIM),
                s[:, :].rearrange("p (o g) -> p o g", o=OUT_DIM),
                fac[:, :].unsqueeze(1).broadcast_to([BL, OUT_DIM, OUT_FS]),
                op=MULT,
            )
            return v

        # =============== pass 1: u_hat + s1 ===============
        # W2 DMAs in 2-group tiles (sync queue), XBD in 4-group tiles
        # (gpsimd queue) — few, large, contiguous transfers.
        ps1 = pB.tile([BL, 1024], f32, tag="acc")
        for gq in range(GRP // 4):
            xbdt = xbd_pool.tile([128, 4, 128], f16, tag="xbd")
            nc.gpsimd.dma_start(
                xbdt[:, :, :].rearrange("p a c -> p (a c)"), XBD[gq, :, :]
            )
            w2t = wp_pool.tile([128, 4096], f16, tag="wp")
            nc.sync.dma_start(w2t[:, :], W2[gq, :, :])
            for gw in range(2):
                for k in range(2):
                    grp = gq * 4 + gw * 2 + k
                    w2s = w2t[:, (gw * 2 + k) * 1024:(gw * 2 + k + 1) * 1024]
                    pu0 = pA.tile([128, 512], f32, tag="mm512")
                    pu1 = pA.tile([128, 512], f32, tag="mm512")
                    for h, pu in enumerate((pu0, pu1)):
                        nc.tensor.matmul(
                            pu[:, :], lhsT=xbdt[:, 2 * gw + k, :],
                            rhs=w2s[:, h * 512:(h + 1) * 512],
                            start=True, stop=True,
                        )
                    for h in range(2):
                        nc.tensor.matmul(
                            ps1[:, h * 512:(h + 1) * 512],
                            lhsT=xsum_sb[:, grp, :],
                            rhs=w2s[:, h * 512:(h + 1) * 512],
                            start=(grp == 0), stop=(grp == GRP - 1),
                        )
                    ug = u_sb[:, grp, :, :].rearrange("p o g -> p (o g)")
                    nc.vector.tensor_copy(ug[:, 0:384], pu0[:, 0:384])
                    nc.scalar.copy(ug[:, 384:512], pu0[:, 384:512])
                    nc.scalar.copy(ug[:, 512:1024], pu1[:, :])

        # =============== iteration 1 (c uniform = 1/64) ===============
        v1 = squash(ps1[:, :], 1.0 / OUT_FS, "v1")
        nc.vector.tensor_copy(vcum[:, :], v1[:, :])

        # =============== iterations 2..T ===============
        for it in (2, 3):
            sfx = f"_i{it}"
            # --- broadcast Vcum to all 128 partitions (fp16) ---
            pv0 = pA.tile([128, 512], f32, tag="mm512")
            pv1 = pA.tile([128, 512], f32, tag="mm512")
            for h, pv in enumerate((pv0, pv1)):
                nc.tensor.matmul(
                    pv[:, :], lhsT=bsel_sb[:, :],
                    rhs=vcum[:, h * 512:(h + 1) * 512],
                    start=True, stop=True,
                )
            nc.vector.tensor_copy(vbc[:, 0:512], pv0[:, :])
            nc.scalar.copy(vbc[:, 512:1024], pv1[:, :])
            vbc3 = vbc[:, :].rearrange("p (o g) -> p o g", o=OUT_DIM)

            # --- phase A: b_k = sum_o u*Vcum  (9 blocks x 8 grps) ---
            vbc4 = vbc3.unsqueeze(1).broadcast_to([128, 4, OUT_DIM, OUT_FS])
            for blk in range(9):
                pbk = pA.tile([128, 512], f32, tag="mm512")
                for q in range(2):
                    bq = blk * 2 + q
                    g0 = blk * 8 + q * 4
                    w4 = wp_pool.tile([128, 4, OUT_DIM, OUT_FS], f16, tag="wp")
                    eng = nc.gpsimd if bq % 4 == 3 else nc.vector
                    eng.tensor_tensor(
                        w4[:, :, :, :], u_sb[:, g0:g0 + 4, :, :], vbc4, op=MULT
                    )
                    for o in range(OUT_DIM):
                        nc.tensor.matmul(
                            pbk[:, q * 256:(q + 1) * 256],
                            lhsT=i128_sb[:, :], rhs=w4[:, :, o, :],
                            start=(o == 0), stop=(o == OUT_DIM - 1),
                        )
                nc.scalar.activation(
                    ec_sb[:, blk * 8:(blk + 1) * 8, :].rearrange(
                        "p a g -> p (a g)"
                    ),
                    pbk[:, :], EXP,
                )

            # --- softmax over g (free minor), chunked by grp-halves so it
            # overlaps phase A's tail and phase B's head ---
            def softmax_chunk(glo, ghi):
                n = ghi - glo
                nc.vector.tensor_reduce(
                    den[:, glo:ghi], ec_sb[:, glo:ghi, :], axis=AX, op=ADD
                )
                nc.vector.reciprocal(rden[:, glo:ghi], den[:, glo:ghi])
                nc.vector.tensor_tensor(
                    ec_sb[:, glo:ghi, :], ec_sb[:, glo:ghi, :],
                    rden[:, glo:ghi].unsqueeze(2).broadcast_to(
                        [128, n, OUT_FS]
                    ),
                    op=MULT,
                )

            # --- phase B: s_k = sum_f c*u (pairs of grps) ---
            ps = pB.tile([BL, 1024], f32, tag="acc")

            def phase_b(q_lo, q_hi):
                for q in range(q_lo, q_hi):
                    pc = wp_pool.tile([128, 4, OUT_DIM, OUT_FS], f16, tag="wp")
                    eng = nc.gpsimd if q % 4 == 3 else nc.vector
                    eng.tensor_tensor(
                        pc[:, :, :, :],
                        u_sb[:, 4 * q:4 * q + 4, :, :],
                        ec_sb[:, 4 * q:4 * q + 4, :].unsqueeze(2).broadcast_to(
                            [128, 4, OUT_DIM, OUT_FS]
                        ),
                        op=MULT,
                    )
                    pcf = pc[:, :, :, :].rearrange("p a o g -> p (a o g)")
                    for h in range(8):
                        nc.tensor.matmul(
                            ps[:, (h % 2) * 512:(h % 2) * 512 + 512],
                            lhsT=sel8_sb[:, :],
                            rhs=pcf[:, h * 512:(h + 1) * 512],
                            start=(q == 0 and h < 2),
                            stop=(q == GRP // 4 - 1 and h >= 6),
                        )

            softmax_chunk(0, 32)
            phase_b(0, 8)
            softmax_chunk(32, GRP)
            phase_b(8, GRP // 4)

            v = squash(ps[:, :], 1.0, "v" + sfx)
            if it < 3:
                nc.vector.tensor_add(vcum[:, :], vcum[:, :], v[:, :])
            else:
                nc.sync.dma_start(VOUT[:, :], v[:, :])

    nc.finalize()
    return nc


def prepare_inputs(x, W):
    """Host-side layout prep. Returns (shared_map, [per-core maps])."""
    f16 = np.float16
    # W2[grp, 8j+i, 64o+g] = W[16grp+j, g, i, o]
    W2 = np.ascontiguousarray(
        W.astype(np.float32).reshape(GRP, 16, OUT_FS, IN_DIM, OUT_DIM)
        .transpose(0, 1, 3, 4, 2).reshape(GRP, 128, 1024)
    ).astype(f16)
    # pack W2 into 4-group DMA tiles: [18, 128, 4096]
    W2 = np.ascontiguousarray(
        W2.reshape(GRP // 4, 4, 128, 1024).transpose(0, 2, 1, 3)
        .reshape(GRP // 4, 128, 4096)
    )
    SEL8 = np.tile(np.eye(BL, dtype=f16), (16, 1))            # [128, 8]
    BSEL = np.tile(np.eye(BL, dtype=f16), (1, 16))            # [8, 128]
    I128 = np.eye(128, dtype=f16)

    shared = {"w2": W2, "sel8": SEL8, "bsel": BSEL, "i128": I128}
    per_core = []
    for ci in range(NCORES):
        xc = np.asarray(x[ci * BL:(ci + 1) * BL], dtype=np.float32)
        xr = xc.transpose(1, 2, 0).reshape(GRP, 16, IN_DIM, BL)  # [grp,j,i,b]
        xbd = np.zeros((GRP, 16, IN_DIM, 16, BL), dtype=f16)
        for j in range(16):
            xbd[:, j, :, j, :] = xr[:, j]
        xbd = xbd.reshape(GRP, 128, 128)
        # pack into 4-group DMA tiles: [18, 128, 512]
        xbd = np.ascontiguousarray(
            xbd.reshape(GRP // 4, 4, 128, 128).transpose(0, 2, 1, 3)
            .reshape(GRP // 4, 128, 512)
        )
        xsum = np.ascontiguousarray(
            xr.transpose(1, 2, 0, 3).reshape(128, GRP, BL)
        ).astype(f16)
        m = dict(shared)
        m["xbd"] = xbd
        m["xsum"] = xsum
        per_core.append(m)
    return per_core


def kernel(x, W):
    from concourse.bass_utils import run_bass_kernel_spmd

    x = np.asarray(x)
    W = np.asarray(W)
    if "nc" not in _PROGRAM_CACHE:
        _PROGRAM_CACHE["nc"] = build_program()
    nc = _PROGRAM_CACHE["nc"]
    in_maps = prepare_inputs(x, W)
    res = run_bass_kernel_spmd(nc, in_maps, list(range(NCORES)))
    outs = []
    for ci in range(NCORES):
        v = np.asarray(res.results[ci]["vout"], dtype=np.float32)
        outs.append(v.reshape(BL, OUT_DIM, OUT_FS).transpose(0, 2, 1))
    return np.concatenate(outs, axis=0).astype(np.float32)
